# revision 37
# baseline (speedup 1.0000x reference)
"""Trainium2 Bass kernel for nn_Attention (B=16, C=8, H=W=512).

Per sample b:
  q = Wq.x + bq   [1,H,W]
  k = Wk.x + bk   [1,H,W]
  v = Wv.x + bv   [C,H,W]
  S[i,j] = sum_w q[i,w] k[j,w]; A = softmax_j(S); out[c,i,w] = sum_j A[i,j] v[c,j,w]

Sharding: data-parallel over batch, 2 samples per core, 8 cores, no collectives.

Structure (refined from the hi/lo baseline):
  - x as bf16 hi/lo (xh, xl); qk conv is the 3-term colgrp-packed bf16 matmul
    (the PE overlaps disjoint column tiles, so these are cheap); v conv bf16.
  - v psum [128,2,512] double-buffered; drains alternate Scalar/Vector; the
    qk drain is emitted before the second v-drain so transposes start sooner.
  - q/k kept f32r; transposes use an f32r identity (1.5 cyc/row vs 2.0 f32),
    two half-groups per w-tile so the first half starts mid-conv.
  - S is one ap=512 f32r matmul per (i-tile, w-tile); softmax as before.
  - A-transpose on the PE in bf16 (dedicated 1-bank psum).
  - out stored bf16 in DMA-flat layout [b, it, i, c, w] (one flat 512KB DMA
    per i-tile); host unpermutes and upcasts; out-scale split Scalar/Vector.
  - sample 1's x DMAs are emitted right after sample 0's conv (pool depth 8)
    so its input is prefetched during sample 0's attention phases; sample 0's
    phase5 is split around sample 1's first conv chunks to keep the in-order
    TensorE queue fed across the phase boundary.
"""

import sys

import numpy as np

B, C, H, W = 16, 8, 512, 512
NCORES = 8
BPC = B // NCORES  # samples per core
P = 128
G = 16  # rows per group (P // C)
NR = H // G  # 32 row-groups per sample
CH = 4  # row-groups per x chunk
NCH = NR // CH  # 8 chunks per sample

_CACHE = {}


def _build():
    if "nc" in _CACHE:
        return _CACHE["nc"]
    sys.path.insert(0, "/opt/trn_rl_repo")
    import concourse.bass as bass
    import concourse.tile as tile
    from concourse import bacc, mybir

    f32 = mybir.dt.float32
    f32r = mybir.dt.float32r
    bf16 = mybir.dt.bfloat16
    AF = mybir.ActivationFunctionType
    AX = mybir.AxisListType

    nc = bacc.Bacc("TRN2", target_bir_lowering=False, debug=False)

    xh_d = nc.declare_dram_parameter("xh", [BPC, P, NR, W], bf16, isOutput=False)
    xl_d = nc.declare_dram_parameter("xl", [BPC, P, NR, W], bf16, isOutput=False)
    wqkh_d = nc.declare_dram_parameter("wqkh", [P, 32], bf16, isOutput=False)
    wqkl_d = nc.declare_dram_parameter("wqkl", [P, 32], bf16, isOutput=False)
    wv_d = nc.declare_dram_parameter("wv", [P, P], bf16, isOutput=False)
    bqk_d = nc.declare_dram_parameter("bqk", [P, 1], f32, isOutput=False)
    bv_d = nc.declare_dram_parameter("bv", [P, 1], f32, isOutput=False)
    idf_d = nc.declare_dram_parameter("identf", [P, P], f32r, isOutput=False)
    idb_d = nc.declare_dram_parameter("identb", [P, P], bf16, isOutput=False)
    out_d = nc.declare_dram_parameter("out", [BPC, 4, P, C, W], bf16, isOutput=True)

    with tile.TileContext(nc) as tc:
        with (
            tc.tile_pool(name="consts", bufs=1) as consts,
            tc.tile_pool(name="xhp", bufs=8) as xh_pool,
            tc.tile_pool(name="xlp", bufs=8) as xl_pool,
            tc.tile_pool(name="vg", bufs=2) as vg_pool,
            tc.tile_pool(name="vplane", bufs=2) as vp_pool,
            tc.tile_pool(name="qksb", bufs=8) as qk_pool,
            tc.tile_pool(name="qkt", bufs=1) as qkt_pool,
            tc.tile_pool(name="esb", bufs=6) as e_pool,
            tc.tile_pool(name="atsb", bufs=2) as at_pool,
            tc.tile_pool(name="osb", bufs=2) as o_pool,
            tc.tile_pool(name="stats", bufs=16) as st_pool,
            tc.tile_pool(name="ps", bufs=3, space="PSUM") as ps_pool,
            tc.tile_pool(name="ps_v", bufs=2, space="PSUM") as psv_pool,
            tc.tile_pool(name="ps_at", bufs=1, space="PSUM") as psat_pool,
        ):
            wqkh = consts.tile([P, 32], bf16)
            nc.scalar.dma_start(wqkh[:], wqkh_d.ap())
            wqkl = consts.tile([P, 32], bf16)
            nc.scalar.dma_start(wqkl[:], wqkl_d.ap())
            wv = consts.tile([P, P], bf16)
            nc.scalar.dma_start(wv[:], wv_d.ap())
            bqk = consts.tile([P, 1], f32)
            nc.scalar.dma_start(bqk[:], bqk_d.ap())
            bv = consts.tile([P, 1], f32)
            nc.scalar.dma_start(bv[:], bv_d.ap())
            idf = consts.tile([P, P], f32r)
            nc.scalar.dma_start(idf[:], idf_d.ap())
            idb = consts.tile([P, P], bf16)
            nc.scalar.dma_start(idb[:], idb_d.ap())

            def phase1_load(b):
                chunks = []
                for ch in range(NCH):
                    xh = xh_pool.tile([P, CH, W], bf16, name="xh")
                    xl = xl_pool.tile([P, CH, W], bf16, name="xl")
                    if ch == 0 and b == 0:
                        for hf in range(2):
                            sl = slice(2 * hf, 2 * hf + 2)
                            dsl = slice(2 * hf, 2 * hf + 2)
                            nc.sync.dma_start(
                                xh[:, sl, :], xh_d.ap()[b][:, dsl, :]
                            )
                            nc.scalar.dma_start(
                                xl[:, sl, :], xl_d.ap()[b][:, dsl, :]
                            )
                    else:
                        nc.sync.dma_start(
                            xh[:], xh_d.ap()[b][:, CH * ch : CH * ch + CH, :]
                        )
                        nc.scalar.dma_start(
                            xl[:], xl_d.ap()[b][:, CH * ch : CH * ch + CH, :]
                        )
                    chunks.append((xh, xl))
                return chunks

            def phase1(b, chunks, lo=0, hi=NCH, st=None):
                # conv: v planes (bf16, +bv) and q/k rows (3-term hi/lo, +bqk)
                if st is None:
                    st = {}
                    st["vplane"] = [
                        vp_pool.tile([P, C, W], bf16, name=f"vplane{i}")
                        for i in range(4)
                    ]
                    st["qk_sb"] = []
                for ch in range(lo, hi):
                    xh, xl = chunks[ch]
                    psqk = ps_pool.tile([P, W], f32, name="ps")
                    vg = vg_pool.tile([P, CH, W], bf16, name="vg")
                    for half in range(2):
                        psv = psv_pool.tile([P, 2, W], f32, name="psv")
                        for m2 in range(2):
                            m = 2 * half + m2
                            nc.tensor.matmul(
                                psv[:, m2, :], wv[:], xh[:, m, :],
                                start=True, stop=True,
                            )
                            for wpart, xpart, stt, sp in (
                                (wqkh, xh, True, False),
                                (wqkh, xl, False, False),
                                (wqkl, xh, False, True),
                            ):
                                nc.tensor.matmul(
                                    psqk[32 * m : 32 * m + 32, :],
                                    wpart[:],
                                    xpart[:, m, :],
                                    start=stt,
                                    stop=sp,
                                    tile_position=(0, 32 * m),
                                    skip_group_check=True,
                                )
                        if half == 0:
                            nc.scalar.activation(
                                vg[:, 0:2, :], psv[:], AF.Identity, bias=bv[:]
                            )
                        else:
                            sb = qk_pool.tile([P, W], f32r)
                            if ch % 2 == 0:
                                nc.vector.tensor_scalar_add(
                                    sb[:], psqk[:], bqk[:]
                                )
                            else:
                                nc.scalar.activation(
                                    sb[:], psqk[:], AF.Identity, bias=bqk[:]
                                )
                            st["qk_sb"].append(sb)
                            nc.vector.tensor_scalar_add(
                                vg[:, 2:4, :], psv[:], bv[:]
                            )
                    # bridge: grouped -> channel-plane (SBUF->SBUF DMA per rg)
                    jt = ch // 2
                    for m in range(CH):
                        sl = 64 * (ch % 2) + G * m
                        beng = nc.scalar if m % 2 == 0 else nc.sync
                        beng.dma_start(
                            st["vplane"][jt][sl : sl + G, :, :], vg[:, m, :]
                        )
                return st

            def phase2(b, st):
                # PE-transpose q/k (f32r) -> qkt[wt] [128=w, 1024=(q i | k j)]
                st["qkt"] = [
                    qkt_pool.tile([P, 2 * W], f32r, name=f"qkt{i}") for i in range(4)
                ]
                for h in range(2):
                    for wt in range(4):
                        pst = ps_pool.tile([P, 4, P], f32r, name="ps")
                        for pg4 in range(4):
                            nc.tensor.transpose(
                                pst[:, pg4, :],
                                st["qk_sb"][4 * h + pg4][:, P * wt : P * wt + P],
                                idf[:],
                            )
                        # de-interleave (pg, m, qk, g) -> (qk, pg, m, g)
                        src = pst[:].rearrange(
                            "p pg (m qk g) -> p pg m qk g", m=4, qk=2, g=G
                        )
                        dst = st["qkt"][wt][:].rearrange(
                            "p (qk pgh pg m g) -> p pgh pg m qk g",
                            qk=2, pgh=2, pg=4, m=4, g=G,
                        )[:, h]
                        if (2 * wt + h) % 2 == 0:
                            nc.scalar.copy(dst, src)
                        else:
                            nc.vector.tensor_copy(dst, src)

            def phase3(b, st):
                # S matmul (f32r, ap=512) + softmax stats
                st["e_sb"] = []
                st["rs"] = []
                qkt = st["qkt"]
                for it in range(4):
                    pss = ps_pool.tile([P, W], f32, name="ps")
                    for wt in range(4):
                        nc.tensor.matmul(
                            pss[:],
                            qkt[wt][:, P * it : P * it + P],
                            qkt[wt][:, W : 2 * W],
                            start=(wt == 0),
                            stop=(wt == 3),
                        )
                    mx = st_pool.tile([P, 1], f32)
                    nc.vector.reduce_max(mx[:], pss[:], axis=AX.X, negate=True)
                    esb = e_pool.tile([P, W], bf16)
                    sm = st_pool.tile([P, 1], f32)
                    nc.scalar.activation(
                        esb[:], pss[:], AF.Exp, bias=mx[:], accum_out=sm[:]
                    )
                    rs = st_pool.tile([P, 1], f32)
                    nc.vector.reciprocal(rs[:], sm[:])
                    st["e_sb"].append(esb)
                    st["rs"].append(rs)

            def phase4(b, st):
                # A-transpose on the PE (bf16), psum shared with ps_t pool
                st["at"] = [
                    at_pool.tile([P, W], bf16, name=f"at{i}") for i in range(4)
                ]
                for jt in range(4):
                    psa = psat_pool.tile([P, W], bf16, name="psa")
                    for it in range(4):
                        nc.tensor.transpose(
                            psa[:, P * it : P * it + P],
                            st["e_sb"][it][:, P * jt : P * jt + P],
                            idb[:],
                        )
                    if jt % 2 == 0:
                        nc.vector.tensor_copy(st["at"][jt][:], psa[:])
                    else:
                        nc.scalar.copy(st["at"][jt][:], psa[:])

            def phase5(b, st, its, clo=0, chi=C, osb_in=None):
                # out matmul + normalize (x1/rowsum) + store bf16 flat
                for it in its:
                    osb = osb_in if osb_in is not None else o_pool.tile(
                        [P, C, W], bf16
                    )
                    for c in range(clo, chi):
                        pso = ps_pool.tile([P, W], f32, name="ps")
                        for jt in range(4):
                            nc.tensor.matmul(
                                pso[:],
                                st["at"][jt][:, P * it : P * it + P],
                                st["vplane"][jt][:, c, :],
                                start=(jt == 0),
                                stop=(jt == 3),
                            )
                        if c % 2 == 0:
                            nc.vector.tensor_scalar_mul(
                                osb[:, c, :], pso[:], st["rs"][it][:]
                            )
                        else:
                            nc.scalar.mul(osb[:, c, :], pso[:], st["rs"][it][:])
                    if chi == C:
                        nc.sync.dma_start(out_d.ap()[b, it], osb[:])
                return osb

            # pipelined emission: sample 1's x is prefetched while sample 0
            # runs its attention phases.
            c0 = phase1_load(0)
            s0 = phase1(0, c0)
            c1 = phase1_load(1)
            phase2(0, s0)
            phase3(0, s0)
            phase4(0, s0)
            phase5(0, s0, [0])
            s1 = phase1(1, c1, 0, 2)
            phase5(0, s0, [1])
            phase1(1, c1, 2, 4, st=s1)
            phase5(0, s0, [2])
            phase1(1, c1, 4, 6, st=s1)
            phase1(1, c1, 6, NCH, st=s1)
            ob3 = phase5(0, s0, [3], 0, 4)
            phase2(1, s1)
            phase5(0, s0, [3], 4, C, osb_in=ob3)
            phase3(1, s1)
            phase4(1, s1)
            phase5(1, s1, [0, 1, 2, 3])

    nc.compile()
    _CACHE["nc"] = nc
    return nc


def _make_consts(Wq, bq, Wk, bk, Wv, bv):
    import ml_dtypes

    wqk = np.zeros((P, 32), np.float32)
    for g in range(G):
        for c in range(C):
            wqk[g * C + c, g] = Wq[0, c]
            wqk[g * C + c, 16 + g] = Wk[0, c]
    wv = np.zeros((P, P), np.float32)
    for g in range(G):
        for ci in range(C):
            for co in range(C):
                wv[g * C + ci, g * C + co] = Wv[co, ci]
    bqk = np.concatenate([np.full(16, bq[0]), np.full(16, bk[0])] * 4).astype(
        np.float32
    )[:, None]
    bvv = np.tile(bv.astype(np.float32), G)[:, None]
    eyef = np.eye(P, dtype=np.float32)
    eyeb = np.eye(P).astype(ml_dtypes.bfloat16)
    wqkh = wqk.astype(ml_dtypes.bfloat16)
    wqkl = (wqk - wqkh.astype(np.float32)).astype(ml_dtypes.bfloat16)
    return (wqkh, wqkl, wv.astype(ml_dtypes.bfloat16), bqk, bvv, eyef, eyeb)


def _split_x(x):
    import ml_dtypes

    x = np.asarray(x, dtype=np.float32)
    xh = x.astype(ml_dtypes.bfloat16)
    xl = (x - xh.astype(np.float32)).astype(ml_dtypes.bfloat16)
    # [B,C,H,W] -> [B, (g c)=128, r=NR, W]   (p = g*C + c, i = r*G + g)
    perm = lambda a: np.ascontiguousarray(
        a.reshape(B, C, NR, G, W).transpose(0, 3, 1, 2, 4).reshape(B, G * C, NR, W)
    )
    return perm(xh), perm(xl)


def _unperm_out(o):
    # [BPC, 4, 128, C, W] bf16 -> [BPC, C, H, W] f32
    return (
        np.asarray(o)
        .astype(np.float32)
        .transpose(0, 3, 1, 2, 4)
        .reshape(BPC, C, H, W)
    )


def kernel(x, Wq, bq, Wk, bk, Wv, bv):
    sys.path.insert(0, "/opt/trn_rl_repo")
    from concourse.bass_utils import run_bass_kernel_spmd

    nc = _build()
    wqkh, wqkl, wv, bqk, bvv, eyef, eyeb = _make_consts(
        np.asarray(Wq), np.asarray(bq), np.asarray(Wk), np.asarray(bk),
        np.asarray(Wv), np.asarray(bv),
    )
    xh, xl = _split_x(x)
    in_maps = []
    for core in range(NCORES):
        in_maps.append(
            {
                "xh": xh[BPC * core : BPC * core + BPC],
                "xl": xl[BPC * core : BPC * core + BPC],
                "wqkh": wqkh,
                "wqkl": wqkl,
                "wv": wv,
                "bqk": bqk,
                "bv": bvv,
                "identf": eyef,
                "identb": eyeb,
            }
        )
    res = run_bass_kernel_spmd(nc, in_maps, core_ids=list(range(NCORES)))
    out = np.concatenate([_unperm_out(r["out"]) for r in res.results], axis=0)
    return out


# revision 38
# speedup vs baseline: 1.0258x; 1.0258x over previous
"""Trainium2 Bass kernel for nn_Attention (B=16, C=8, H=W=512).

Per sample b:
  q = Wq.x + bq   [1,H,W]
  k = Wk.x + bk   [1,H,W]
  v = Wv.x + bv   [C,H,W]
  S[i,j] = sum_w q[i,w] k[j,w]; A = softmax_j(S); out[c,i,w] = sum_j A[i,j] v[c,j,w]

Sharding: data-parallel over batch, 2 samples per core, 8 cores, no collectives.

Structure (refined from the hi/lo baseline):
  - x as bf16 hi/lo (xh, xl); qk conv is the 3-term colgrp-packed bf16 matmul
    (the PE overlaps disjoint column tiles, so these are cheap); v conv bf16.
  - v psum [128,2,512] double-buffered; drains alternate Scalar/Vector; the
    qk drain is emitted before the second v-drain so transposes start sooner.
  - q/k kept f32r; transposes use an f32r identity (1.5 cyc/row vs 2.0 f32),
    two half-groups per w-tile so the first half starts mid-conv.
  - S is one ap=512 f32r matmul per (i-tile, w-tile); softmax as before.
  - A-transpose on the PE in bf16 (dedicated 1-bank psum).
  - out stored bf16 in DMA-flat layout [b, it, i, c, w] (one flat 512KB DMA
    per i-tile); host unpermutes and upcasts; out-scale split Scalar/Vector.
  - sample 1's x DMAs are emitted right after sample 0's conv (pool depth 8)
    so its input is prefetched during sample 0's attention phases; sample 0's
    phase5 is split around sample 1's first conv chunks to keep the in-order
    TensorE queue fed across the phase boundary.
"""

import sys

import numpy as np

B, C, H, W = 16, 8, 512, 512
NCORES = 8
BPC = B // NCORES  # samples per core
P = 128
G = 16  # rows per group (P // C)
NR = H // G  # 32 row-groups per sample
CH = 4  # row-groups per x chunk
NCH = NR // CH  # 8 chunks per sample

_CACHE = {}


def _build():
    if "nc" in _CACHE:
        return _CACHE["nc"]
    sys.path.insert(0, "/opt/trn_rl_repo")
    import concourse.bass as bass
    import concourse.tile as tile
    from concourse import bacc, mybir

    f32 = mybir.dt.float32
    f32r = mybir.dt.float32r
    bf16 = mybir.dt.bfloat16
    AF = mybir.ActivationFunctionType
    AX = mybir.AxisListType

    nc = bacc.Bacc("TRN2", target_bir_lowering=False, debug=False)

    xh_d = nc.declare_dram_parameter("xh", [BPC, P, NR, W], bf16, isOutput=False)
    xl_d = nc.declare_dram_parameter("xl", [BPC, P, NR, W], bf16, isOutput=False)
    wqkh_d = nc.declare_dram_parameter("wqkh", [P, 32], bf16, isOutput=False)
    wqkl_d = nc.declare_dram_parameter("wqkl", [P, 32], bf16, isOutput=False)
    wv_d = nc.declare_dram_parameter("wv", [P, P], bf16, isOutput=False)
    bqk_d = nc.declare_dram_parameter("bqk", [P, 1], f32, isOutput=False)
    bv_d = nc.declare_dram_parameter("bv", [P, 1], f32, isOutput=False)
    idf_d = nc.declare_dram_parameter("identf", [P, P], f32r, isOutput=False)
    idb_d = nc.declare_dram_parameter("identb", [P, P], bf16, isOutput=False)
    out_d = nc.declare_dram_parameter("out", [BPC, 4, P, C, W], bf16, isOutput=True)

    with tile.TileContext(nc) as tc:
        with (
            tc.tile_pool(name="consts", bufs=1) as consts,
            tc.tile_pool(name="xhp", bufs=8) as xh_pool,
            tc.tile_pool(name="xlp", bufs=8) as xl_pool,
            tc.tile_pool(name="vg", bufs=2) as vg_pool,
            tc.tile_pool(name="vplane", bufs=2) as vp_pool,
            tc.tile_pool(name="qksb", bufs=8) as qk_pool,
            tc.tile_pool(name="qkt", bufs=1) as qkt_pool,
            tc.tile_pool(name="esb", bufs=6) as e_pool,
            tc.tile_pool(name="atsb", bufs=2) as at_pool,
            tc.tile_pool(name="osb", bufs=2) as o_pool,
            tc.tile_pool(name="stats", bufs=16) as st_pool,
            tc.tile_pool(name="ps", bufs=3, space="PSUM") as ps_pool,
            tc.tile_pool(name="ps_v", bufs=2, space="PSUM") as psv_pool,
            tc.tile_pool(name="ps_at", bufs=1, space="PSUM") as psat_pool,
        ):
            wqkh = consts.tile([P, 32], bf16)
            nc.scalar.dma_start(wqkh[:], wqkh_d.ap())
            wqkl = consts.tile([P, 32], bf16)
            nc.scalar.dma_start(wqkl[:], wqkl_d.ap())
            wv = consts.tile([P, P], bf16)
            nc.scalar.dma_start(wv[:], wv_d.ap())
            bqk = consts.tile([P, 1], f32)
            nc.scalar.dma_start(bqk[:], bqk_d.ap())
            bv = consts.tile([P, 1], f32)
            nc.scalar.dma_start(bv[:], bv_d.ap())
            idf = consts.tile([P, P], f32r)
            nc.scalar.dma_start(idf[:], idf_d.ap())
            idb = consts.tile([P, P], bf16)
            nc.scalar.dma_start(idb[:], idb_d.ap())

            def phase1_load(b):
                chunks = []
                for ch in range(NCH):
                    xh = xh_pool.tile([P, CH, W], bf16, name="xh")
                    xl = xl_pool.tile([P, CH, W], bf16, name="xl")
                    if ch == 0 and b == 0:
                        for hf in range(2):
                            sl = slice(2 * hf, 2 * hf + 2)
                            dsl = slice(2 * hf, 2 * hf + 2)
                            nc.sync.dma_start(
                                xh[:, sl, :], xh_d.ap()[b][:, dsl, :]
                            )
                            nc.scalar.dma_start(
                                xl[:, sl, :], xl_d.ap()[b][:, dsl, :]
                            )
                    else:
                        nc.sync.dma_start(
                            xh[:], xh_d.ap()[b][:, CH * ch : CH * ch + CH, :]
                        )
                        nc.scalar.dma_start(
                            xl[:], xl_d.ap()[b][:, CH * ch : CH * ch + CH, :]
                        )
                    chunks.append((xh, xl))
                return chunks

            def phase1(b, chunks, lo=0, hi=NCH, st=None):
                # conv: v planes (bf16, +bv) and q/k rows (3-term hi/lo, +bqk)
                if st is None:
                    st = {}
                    st["vplane"] = [
                        vp_pool.tile([P, C, W], bf16, name=f"vplane{i}")
                        for i in range(4)
                    ]
                    st["qk_sb"] = []
                for ch in range(lo, hi):
                    xh, xl = chunks[ch]
                    psqk = ps_pool.tile([P, W], f32, name="ps")
                    vg = vg_pool.tile([P, CH, W], bf16, name="vg")
                    for half in range(2):
                        psv = psv_pool.tile([P, 2, W], f32, name="psv")
                        for m2 in range(2):
                            m = 2 * half + m2
                            nc.tensor.matmul(
                                psv[:, m2, :], wv[:], xh[:, m, :],
                                start=True, stop=True,
                            )
                            for wpart, xpart, stt, sp in (
                                (wqkh, xh, True, False),
                                (wqkh, xl, False, False),
                                (wqkl, xh, False, True),
                            ):
                                nc.tensor.matmul(
                                    psqk[32 * m : 32 * m + 32, :],
                                    wpart[:],
                                    xpart[:, m, :],
                                    start=stt,
                                    stop=sp,
                                    tile_position=(0, 32 * m),
                                    skip_group_check=True,
                                )
                        if half == 0:
                            nc.scalar.activation(
                                vg[:, 0:2, :], psv[:], AF.Identity, bias=bv[:]
                            )
                        else:
                            sb = qk_pool.tile([P, W], f32r)
                            if ch % 2 == 0:
                                nc.vector.tensor_scalar_add(
                                    sb[:], psqk[:], bqk[:]
                                )
                            else:
                                nc.scalar.activation(
                                    sb[:], psqk[:], AF.Identity, bias=bqk[:]
                                )
                            st["qk_sb"].append(sb)
                            nc.vector.tensor_scalar_add(
                                vg[:, 2:4, :], psv[:], bv[:]
                            )
                    # bridge: grouped -> channel-plane (SBUF->SBUF DMA per rg)
                    jt = ch // 2
                    for m in range(CH):
                        sl = 64 * (ch % 2) + G * m
                        beng = nc.scalar if m % 2 == 0 else nc.sync
                        beng.dma_start(
                            st["vplane"][jt][sl : sl + G, :, :], vg[:, m, :]
                        )
                return st

            def phase2(b, st):
                # PE-transpose q/k (f32r) -> qkt[wt] [128=w, 1024=(q i | k j)]
                st["qkt"] = [
                    qkt_pool.tile([P, 2 * W], f32r, name=f"qkt{i}") for i in range(4)
                ]
                for h in range(2):
                    for wt in range(4):
                        pst = ps_pool.tile([P, 4, P], f32r, name="ps")
                        for pg4 in range(4):
                            nc.tensor.transpose(
                                pst[:, pg4, :],
                                st["qk_sb"][4 * h + pg4][:, P * wt : P * wt + P],
                                idf[:],
                            )
                        # de-interleave (pg, m, qk, g) -> (qk, pg, m, g)
                        src = pst[:].rearrange(
                            "p pg (m qk g) -> p pg m qk g", m=4, qk=2, g=G
                        )
                        dst = st["qkt"][wt][:].rearrange(
                            "p (qk pgh pg m g) -> p pgh pg m qk g",
                            qk=2, pgh=2, pg=4, m=4, g=G,
                        )[:, h]
                        if (2 * wt + h) % 2 == 0:
                            nc.scalar.copy(dst, src)
                        else:
                            nc.vector.tensor_copy(dst, src)

            def phase3(b, st):
                # S matmul (f32r, ap=512) + softmax stats
                st["e_sb"] = []
                st["rs"] = []
                qkt = st["qkt"]
                for it in range(4):
                    pss = ps_pool.tile([P, W], f32, name="ps")
                    for wt in range(4):
                        nc.tensor.matmul(
                            pss[:],
                            qkt[wt][:, P * it : P * it + P],
                            qkt[wt][:, W : 2 * W],
                            start=(wt == 0),
                            stop=(wt == 3),
                        )
                    mx = st_pool.tile([P, 1], f32)
                    nc.vector.reduce_max(mx[:], pss[:], axis=AX.X, negate=True)
                    esb = e_pool.tile([P, W], bf16)
                    sm = st_pool.tile([P, 1], f32)
                    nc.scalar.activation(
                        esb[:], pss[:], AF.Exp, bias=mx[:], accum_out=sm[:]
                    )
                    rs = st_pool.tile([P, 1], f32)
                    nc.vector.reciprocal(rs[:], sm[:])
                    st["e_sb"].append(esb)
                    st["rs"].append(rs)

            def phase4(b, st):
                # A-transpose on the PE (bf16), psum shared with ps_t pool
                st["at"] = [
                    at_pool.tile([P, W], bf16, name=f"at{i}") for i in range(4)
                ]
                for jt in range(4):
                    psa = psat_pool.tile([P, W], bf16, name="psa")
                    for it in range(4):
                        nc.tensor.transpose(
                            psa[:, P * it : P * it + P],
                            st["e_sb"][it][:, P * jt : P * jt + P],
                            idb[:],
                        )
                    if jt % 2 == 0:
                        nc.vector.tensor_copy(st["at"][jt][:], psa[:])
                    else:
                        nc.scalar.copy(st["at"][jt][:], psa[:])

            def phase5(b, st, its, clo=0, chi=C, osb_in=None):
                # out matmul + normalize (x1/rowsum) + store bf16 flat
                for it in its:
                    osb = osb_in if osb_in is not None else o_pool.tile(
                        [P, C, W], bf16
                    )
                    for c in range(clo, chi):
                        pso = ps_pool.tile([P, W], f32, name="ps")
                        for jt in range(4):
                            nc.tensor.matmul(
                                pso[:],
                                st["at"][jt][:, P * it : P * it + P],
                                st["vplane"][jt][:, c, :],
                                start=(jt == 0),
                                stop=(jt == 3),
                            )
                        if c % 2 == 0:
                            nc.vector.tensor_scalar_mul(
                                osb[:, c, :], pso[:], st["rs"][it][:]
                            )
                        else:
                            nc.scalar.mul(osb[:, c, :], pso[:], st["rs"][it][:])
                    if chi == C:
                        nc.sync.dma_start(out_d.ap()[b, it], osb[:])
                return osb

            # pipelined emission: sample 1's x is prefetched while sample 0
            # runs its attention phases.
            c0 = phase1_load(0)
            s0 = phase1(0, c0)
            c1 = phase1_load(1)
            phase2(0, s0)
            phase3(0, s0)
            phase4(0, s0)
            phase5(0, s0, [0])
            s1 = phase1(1, c1, 0, 2)
            phase5(0, s0, [1])
            phase1(1, c1, 2, 4, st=s1)
            phase5(0, s0, [2])
            phase1(1, c1, 4, 6, st=s1)
            phase5(0, s0, [3])
            phase1(1, c1, 6, NCH, st=s1)
            phase2(1, s1)
            phase3(1, s1)
            phase4(1, s1)
            phase5(1, s1, [0, 1, 2, 3])

    nc.compile()
    _CACHE["nc"] = nc
    return nc


def _make_consts(Wq, bq, Wk, bk, Wv, bv):
    import ml_dtypes

    wqk = np.zeros((P, 32), np.float32)
    for g in range(G):
        for c in range(C):
            wqk[g * C + c, g] = Wq[0, c]
            wqk[g * C + c, 16 + g] = Wk[0, c]
    wv = np.zeros((P, P), np.float32)
    for g in range(G):
        for ci in range(C):
            for co in range(C):
                wv[g * C + ci, g * C + co] = Wv[co, ci]
    bqk = np.concatenate([np.full(16, bq[0]), np.full(16, bk[0])] * 4).astype(
        np.float32
    )[:, None]
    bvv = np.tile(bv.astype(np.float32), G)[:, None]
    eyef = np.eye(P, dtype=np.float32)
    eyeb = np.eye(P).astype(ml_dtypes.bfloat16)
    wqkh = wqk.astype(ml_dtypes.bfloat16)
    wqkl = (wqk - wqkh.astype(np.float32)).astype(ml_dtypes.bfloat16)
    return (wqkh, wqkl, wv.astype(ml_dtypes.bfloat16), bqk, bvv, eyef, eyeb)


def _split_x(x):
    import ml_dtypes

    x = np.asarray(x, dtype=np.float32)
    xh = x.astype(ml_dtypes.bfloat16)
    xl = (x - xh.astype(np.float32)).astype(ml_dtypes.bfloat16)
    # [B,C,H,W] -> [B, (g c)=128, r=NR, W]   (p = g*C + c, i = r*G + g)
    perm = lambda a: np.ascontiguousarray(
        a.reshape(B, C, NR, G, W).transpose(0, 3, 1, 2, 4).reshape(B, G * C, NR, W)
    )
    return perm(xh), perm(xl)


def _unperm_out(o):
    # [BPC, 4, 128, C, W] bf16 -> [BPC, C, H, W] f32
    return (
        np.asarray(o)
        .astype(np.float32)
        .transpose(0, 3, 1, 2, 4)
        .reshape(BPC, C, H, W)
    )


def kernel(x, Wq, bq, Wk, bk, Wv, bv):
    sys.path.insert(0, "/opt/trn_rl_repo")
    from concourse.bass_utils import run_bass_kernel_spmd

    nc = _build()
    wqkh, wqkl, wv, bqk, bvv, eyef, eyeb = _make_consts(
        np.asarray(Wq), np.asarray(bq), np.asarray(Wk), np.asarray(bk),
        np.asarray(Wv), np.asarray(bv),
    )
    xh, xl = _split_x(x)
    in_maps = []
    for core in range(NCORES):
        in_maps.append(
            {
                "xh": xh[BPC * core : BPC * core + BPC],
                "xl": xl[BPC * core : BPC * core + BPC],
                "wqkh": wqkh,
                "wqkl": wqkl,
                "wv": wv,
                "bqk": bqk,
                "bv": bvv,
                "identf": eyef,
                "identb": eyeb,
            }
        )
    res = run_bass_kernel_spmd(nc, in_maps, core_ids=list(range(NCORES)))
    out = np.concatenate([_unperm_out(r["out"]) for r in res.results], axis=0)
    return out
